# revision 19
# baseline (speedup 1.0000x reference)
# Trainium2 Bass kernel for nn_CFTAuxHead (bilinear 4x resize + bbox
# rasterization + MSE loss), data-parallel over batch across 8 NeuronCores.
#
# Math summary (per sample):
#   feat_up = A^T @ F @ A  (A = exact 160->640 bilinear weight matrix, fp16)
#   heatmap = last-writer-wins paint of 128 axis-aligned rects (value z_n)
#   loss    = mean((feat_up - heatmap)^2) over all pixels
#
# Rasterization: 2 paint matmuls per row-tile over box indicators with
# per-box weights w_n = 2^(n-65) (exponent encode, single group):
#   C  = sum_n w_n [covered] + eps      CA = sum_n (-z_n) w_n [covered]
# Per-pixel decode (bf16, exact when coverage depth <= 1, ~exact depth 2):
#   E2 = 2^(top exponent + 1) via int16 bit trick on C
#   den = E2 - C  (Sterbenz-exact);  Zneg = clamp(CA / den, -2, 2) = -z_top
# The -z map is then accumulated into the feat PSUM tile via an identity
# matmul, so (feat_up - z) forms in PSUM and the Act engine squares +
# accumulates it into the loss in one op.

import os
import numpy as np

B, C_IN, H, W = 32, 1, 160, 160
UP = 4
HO, WO = H * UP, W * UP
NBOX = 128
NCORES = 8
SPC = B // NCORES  # samples per core
NPIX = float(B * HO * WO)

_CACHE = {}


def _resize_matrix():
    """Exact bilinear (half-pixel centers, edge-clamped) 160->640 matrix,
    matching jax.image.resize(method='bilinear') for upsampling.
    All entries are multiples of 1/8 -> exact in fp16."""
    n_in, n_out = H, HO
    scale = n_out / n_in
    x = (np.arange(n_out, dtype=np.float64) + 0.5) / scale - 0.5
    k = np.arange(n_in, dtype=np.float64)
    w = np.maximum(0.0, 1.0 - np.abs(x[None, :] - k[:, None]))  # [in, out]
    w = w / w.sum(axis=0, keepdims=True)
    return w.astype(np.float16)


def _build(krep=1):
    import concourse.bacc as bacc
    import concourse.mybir as mybir
    from concourse.tile import TileContext

    fp32 = mybir.dt.float32
    bf16 = mybir.dt.bfloat16
    fp16 = mybir.dt.float16
    i16 = mybir.dt.int16
    u16 = mybir.dt.uint16
    i32 = mybir.dt.int32
    Alu = mybir.AluOpType
    ActF = mybir.ActivationFunctionType

    nc = bacc.Bacc("TRN2", target_bir_lowering=False, debug=False,
                   enable_asserts=False, num_devices=NCORES)
    feat_d = nc.dram_tensor("feat", [SPC, H, W], fp16, kind="ExternalInput")
    box_d = nc.dram_tensor("boxes", [SPC, NBOX, 5], fp32, kind="ExternalInput")
    amat_d = nc.dram_tensor("amat", [H, HO], fp16, kind="ExternalInput")
    iden_d = nc.dram_tensor("iden", [128, 128], fp32, kind="ExternalInput")
    out_d = nc.dram_tensor("out", [1, 1], fp32, kind="ExternalOutput")
    debug = os.environ.get("KV_DEBUG", "0") == "1"
    dbg_d = {}
    if debug:
        for nm in ("c16", "e2", "den", "rden", "z", "zc", "tf", "u", "v0",
                   "vs", "va", "ts", "ta", "tx", "ty"):
            dbg_d[nm] = nc.dram_tensor(f"dbg_{nm}", [128, HO], fp32,
                                       kind="ExternalOutput")

    EPS = float(2.0 ** -94)
    BANKS = (slice(0, 512), slice(512, 640))

    with TileContext(nc, num_cores=NCORES) as tc:
        with tc.tile_pool(name="const", bufs=1) as cpool, \
             tc.tile_pool(name="samp", bufs=2) as spool, \
             tc.tile_pool(name="dec", bufs=2) as dpool, \
             tc.tile_pool(name="psS", bufs=1, space="PSUM") as psS, \
             tc.tile_pool(name="psA", bufs=2, space="PSUM") as psA, \
             tc.tile_pool(name="psF", bufs=1, space="PSUM") as psF:

            # ---- constants ----
            A0h = cpool.tile([128, HO], fp16, tag="A0h")
            A1h = cpool.tile([32, HO], fp16, tag="A1h")
            nc.sync.dma_start(A0h[:], amat_d.ap()[0:128, :])
            nc.sync.dma_start(A1h[:], amat_d.ap()[128:160, :])

            iden32 = cpool.tile([128, 128], fp32, tag="iden32")
            nc.sync.dma_start(iden32[:], iden_d.ap())
            idbf = cpool.tile([128, 128], bf16, tag="idbf")
            nc.vector.tensor_copy(idbf[:], iden32[:])

            iota_32 = cpool.tile([128, HO], i32, tag="iot32")
            nc.gpsimd.iota(iota_32[:], pattern=[[1, HO]], base=0,
                           channel_multiplier=0)
            iota16 = cpool.tile([128, HO], i16, tag="iot16")
            nc.vector.tensor_copy(iota16[:], iota_32[:])

            nidx_i = cpool.tile([128, 1], i32, tag="nidxi")
            nc.gpsimd.iota(nidx_i[:], pattern=[[1, 1]], base=1,
                           channel_multiplier=1)  # n' = n+1 in 1..128
            w2b = cpool.tile([128, 1], i32, tag="w2b")
            nc.vector.tensor_scalar(w2b[:], nidx_i[:], 62, None, Alu.add)
            nc.vector.tensor_scalar(w2b[:], w2b[:], 23, None,
                                    Alu.logical_shift_left)
            # w2n = 2^(n'-65), exact in bf16; view as fp32 for ptr use
            w2n = w2b[:].bitcast(fp32)

            eps_t = cpool.tile([128, 1], fp32, tag="epsb")
            nc.vector.memset(eps_t[:], EPS)
            ones_t = cpool.tile([128, 1], fp32, tag="ones")
            nc.vector.memset(ones_t[:], 1.0)

            accbuf = cpool.tile([128, krep * SPC * 5], fp32, tag="acc")

            for rep in range(krep):
                for s in range(SPC):
                    # ---- loads ----
                    F0 = spool.tile([128, W], fp16, tag="F0")
                    F1 = spool.tile([32, W], fp16, tag="F1")
                    nc.sync.dma_start(F0[:], feat_d.ap()[s, 0:128, :])
                    nc.sync.dma_start(F1[:], feat_d.ap()[s, 128:160, :])
                    bx = spool.tile([128, 5], fp32, tag="bx")
                    nc.sync.dma_start(bx[:], box_d.ap()[s])
                    xq = bx[:, 0:1]
                    yq = bx[:, 1:2]
                    zq = bx[:, 2:3]
                    wq = bx[:, 3:4]
                    lq = bx[:, 4:5]

                    # ---- box prep (all [128,1]) ----
                    # HW float->int convert rounds to nearest, so floor(v)
                    # is computed as convert(v - 0.5).
                    cxi = dpool.tile([128, 1], i16, tag="cxi")
                    nc.vector.tensor_scalar(cxi[:], xq, -0.5, None, Alu.add)
                    cyi = dpool.tile([128, 1], i16, tag="cyi")
                    nc.vector.tensor_scalar(cyi[:], yq, -0.5, None, Alu.add)
                    # h = max(floor(w/2), 3) = round(max(w*0.5 - 0.5, 2.6));
                    # 2.6 not 2.5: round-half-even(2.5) = 2 would break MIN_RADIUS
                    hwf = dpool.tile([128, 1], fp32, tag="hwf")
                    nc.vector.tensor_scalar(hwf[:], wq, 0.5, -0.5, Alu.mult,
                                            Alu.add)
                    hwi = dpool.tile([128, 1], i16, tag="hwi")
                    nc.vector.tensor_scalar(hwi[:], hwf[:], 2.6, None, Alu.max)
                    hlf = dpool.tile([128, 1], fp32, tag="hlf")
                    nc.vector.tensor_scalar(hlf[:], lq, 0.5, -0.5, Alu.mult,
                                            Alu.add)
                    hli = dpool.tile([128, 1], i16, tag="hli")
                    nc.vector.tensor_scalar(hli[:], hlf[:], 2.6, None, Alu.max)
                    xmini = dpool.tile([128, 1], i16, tag="xmini")
                    nc.vector.tensor_tensor(xmini[:], cxi[:], hwi[:],
                                            Alu.subtract)
                    ymini = dpool.tile([128, 1], i16, tag="ymini")
                    nc.vector.tensor_tensor(ymini[:], cyi[:], hli[:],
                                            Alu.subtract)
                    # len = xmax - xmin = 2h + 1
                    lenxi = dpool.tile([128, 1], i16, tag="lenxi")
                    nc.vector.tensor_scalar(lenxi[:], hwi[:], 2, 1, Alu.mult,
                                            Alu.add)
                    lenyi = dpool.tile([128, 1], i16, tag="lenyi")
                    nc.vector.tensor_scalar(lenyi[:], hli[:], 2, 1, Alu.mult,
                                            Alu.add)
                    # fp32 views of the per-box scalars (scalar-ptr operands
                    # must be fp32)
                    xminf = dpool.tile([128, 1], fp32, tag="xminf")
                    nc.vector.tensor_copy(xminf[:], xmini[:])
                    yminf = dpool.tile([128, 1], fp32, tag="yminf")
                    nc.vector.tensor_copy(yminf[:], ymini[:])
                    lenxf = dpool.tile([128, 1], fp32, tag="lenxf")
                    nc.vector.tensor_copy(lenxf[:], lenxi[:])
                    lenyf = dpool.tile([128, 1], fp32, tag="lenyf")
                    nc.vector.tensor_copy(lenyf[:], lenyi[:])
                    # validity and paint weights
                    vw = dpool.tile([128, 1], fp32, tag="vw")
                    nc.vector.tensor_scalar(vw[:], wq, 0.0, None, Alu.is_gt)
                    vv = dpool.tile([128, 1], fp32, tag="vv")
                    nc.vector.scalar_tensor_tensor(vv[:], lq, 0.0, vw[:],
                                                   Alu.is_gt, Alu.logical_and)
                    wsv = dpool.tile([128, 1], fp32, tag="wsv")
                    nc.vector.tensor_tensor(wsv[:], w2n, vv[:], Alu.mult)
                    wav = dpool.tile([128, 1], fp32, tag="wav")
                    nc.vector.scalar_tensor_tensor(wav[:], zq, -1.0, wsv[:],
                                                   Alu.mult, Alu.mult)

                    # ---- U (row indicator) / V (col) in bf16 via int16 ----
                    tx = spool.tile([128, HO], i16, tag="tx")
                    nc.vector.tensor_scalar(tx[:], iota16[:], xminf[:], None,
                                            Alu.subtract)
                    U = spool.tile([128, HO], bf16, tag="U")
                    nc.vector.tensor_scalar(U[:], tx[:].bitcast(u16),
                                            lenxf[:], None, Alu.is_lt)
                    ty = spool.tile([128, HO], i16, tag="ty")
                    nc.vector.tensor_scalar(ty[:], iota16[:], yminf[:], None,
                                            Alu.subtract)
                    V0 = spool.tile([128, HO], bf16, tag="V0")
                    nc.vector.tensor_scalar(V0[:], ty[:].bitcast(u16),
                                            lenyf[:], None, Alu.is_lt)
                    Vs = spool.tile([128, HO], bf16, tag="Vs")
                    nc.vector.tensor_scalar(Vs[:], V0[:], wsv[:], None,
                                            Alu.mult)
                    Va = spool.tile([128, HO], bf16, tag="Va")
                    nc.vector.tensor_scalar(Va[:], V0[:], wav[:], None,
                                            Alu.mult)

                    # ---- step1: out1 = F^T A (row resize), fp16 ----
                    out1a = spool.tile([128, HO], fp16, tag="out1a")
                    out1b = spool.tile([32, HO], fp16, tag="out1b")
                    for part, (msz, moff, o1) in enumerate(
                            [(128, 0, out1a), (32, 128, out1b)]):
                        p1 = psS.tile([128, HO], fp32, tag="Ts")
                        for hs in BANKS:
                            nc.tensor.matmul(p1[0:msz, hs],
                                             F0[:, moff:moff + msz],
                                             A0h[:, hs], start=True,
                                             stop=False)
                            nc.tensor.matmul(p1[0:msz, hs],
                                             F1[:, moff:moff + msz],
                                             A1h[:, hs], start=False,
                                             stop=True)
                        nc.scalar.copy(o1[:], p1[0:msz, :])

                    # ---- per row-tile: paints + decode + loss ----
                    for m in range(5):
                        ms = slice(m * 128, (m + 1) * 128)
                        idx = ((rep * SPC + s) * 5) + m

                        Ts = psS.tile([128, HO], fp32, tag="Ts")
                        Ta = psA.tile([128, HO], fp32, tag="Ta")
                        Tf = psF.tile([128, HO], fp32, tag="Tf")
                        for hs in BANKS:
                            nc.tensor.matmul(Ts[:, hs], U[:, ms], Vs[:, hs],
                                             start=True, stop=True)
                            nc.tensor.matmul(Ta[:, hs], U[:, ms], Va[:, hs],
                                             start=True, stop=True)

                        # decode: C16 on Act (frees Ts)
                        C16 = dpool.tile([128, HO], bf16, tag="C16")
                        nc.scalar.activation(C16[:], Ts[:], ActF.Identity,
                                             bias=eps_t[:], scale=1.0)

                        E1 = dpool.tile([128, HO], i16, tag="E1")
                        nc.vector.tensor_scalar(E1[:], C16[:].bitcast(i16),
                                                -128, None, Alu.bitwise_and)
                        E2 = dpool.tile([128, HO], i16, tag="E2")
                        nc.vector.tensor_scalar(E2[:], E1[:], 128, None,
                                                Alu.add)
                        den = dpool.tile([128, HO], bf16, tag="den")
                        nc.vector.tensor_tensor(den[:], E2[:].bitcast(bf16),
                                                C16[:], Alu.subtract)
                        rden = dpool.tile([128, HO], fp32, tag="rden")
                        nc.vector.reciprocal(rden[:], den[:])
                        # multiply straight from the A-paint PSUM (frees Ta)
                        Z = dpool.tile([128, HO], bf16, tag="Z")
                        nc.vector.tensor_tensor(Z[:], Ta[:], rden[:],
                                                Alu.mult)
                        Zc = dpool.tile([128, HO], bf16, tag="Zc")
                        nc.gpsimd.tensor_scalar(Zc[:], Z[:], -2.0, 2.0,
                                                Alu.max, Alu.min)

                        # feat resize + (-z) paint: Tf := feat_up - z.
                        # Each bank's accumulation group is contiguous.
                        for hs in BANKS:
                            nc.tensor.matmul(Tf[:, hs], out1a[:, ms],
                                             A0h[:, hs], start=True,
                                             stop=False)
                            nc.tensor.matmul(Tf[:, hs], out1b[:, ms],
                                             A1h[:, hs], start=False,
                                             stop=False)
                            nc.tensor.matmul(Tf[:, hs], idbf[:], Zc[:, hs],
                                             start=False, stop=True)

                        # loss: square + accumulate straight from PSUM
                        dsq = dpool.tile([128, HO], bf16, tag="dsq")
                        nc.scalar.activation(
                            dsq[:], Tf[:], ActF.Square,
                            accum_out=accbuf[:, idx:idx + 1])

                        if debug and rep == 0 and s == 0 and m == 0:
                            def dump(nm, ap):
                                t = cpool.tile([128, HO], fp32, tag=f"dbg{nm}")
                                nc.vector.tensor_copy(t[:], ap)
                                nc.sync.dma_start(dbg_d[nm].ap(), t[:])
                            dump("c16", C16[:])
                            dump("e2", E2[:])
                            dump("den", den[:])
                            dump("rden", rden[:])
                            dump("z", Z[:])
                            dump("zc", Zc[:])
                            dump("tf", Tf[:])
                            dump("u", U[:])
                            dump("v0", V0[:])
                            dump("vs", Vs[:])
                            dump("va", Va[:])
                            dump("ts", Ts[:])
                            dump("ta", Ta[:])
                            dump("tx", tx[:])
                            dump("ty", ty[:])

            # ---- final reduction ----
            tot = cpool.tile([128, 1], fp32, tag="tot")
            nc.vector.tensor_reduce(
                tot[:], accbuf[:, 0:krep * SPC * 5],
                mybir.AxisListType.X, Alu.add)
            if krep > 1:
                nc.vector.tensor_scalar(tot[:], tot[:], 1.0 / krep, None,
                                        Alu.mult)
            pfin = psA.tile([128, HO], fp32, tag="Ta")
            nc.tensor.matmul(pfin[0:1, 0:1], tot[:], ones_t[:],
                             start=True, stop=True)
            res = cpool.tile([1, 1], fp32, tag="res")
            nc.scalar.copy(res[:], pfin[0:1, 0:1])
            nc.sync.dma_start(out_d.ap(), res[:])

    nc.compile()
    return nc


def _get_nc(krep=1):
    key = ("nc", krep)
    if key not in _CACHE:
        _CACHE[key] = _build(krep)
    return _CACHE[key]


def run_cores(feat, gt_bboxes, krep=1):
    """Run the SPMD kernel; returns list of per-core sum-of-squared-diffs."""
    from concourse.bass_utils import run_bass_kernel_spmd
    nc = _get_nc(krep)
    amat = _resize_matrix()
    iden = np.eye(128, dtype=np.float32)
    feat = np.asarray(feat, dtype=np.float32)
    gt = np.ascontiguousarray(np.asarray(gt_bboxes, dtype=np.float32))
    feat16 = feat.astype(np.float16)
    in_maps = []
    for i in range(NCORES):
        sl = slice(i * SPC, (i + 1) * SPC)
        in_maps.append({
            "feat": np.ascontiguousarray(feat16[sl, 0]),
            "boxes": np.ascontiguousarray(gt[sl]),
            "amat": amat,
            "iden": iden,
        })
    res = run_bass_kernel_spmd(nc, in_maps, core_ids=list(range(NCORES)))
    return [float(res.results[i]["out"][0, 0]) for i in range(NCORES)]


def kernel(feat, gt_bboxes):
    parts = run_cores(feat, gt_bboxes, krep=1)
    total = float(np.sum(np.asarray(parts, dtype=np.float64)))
    return np.asarray(np.float32(total / NPIX))


# revision 20
# speedup vs baseline: 1.0325x; 1.0325x over previous
# Trainium2 Bass kernel for nn_CFTAuxHead (bilinear 4x resize + bbox
# rasterization + MSE loss), data-parallel over batch across 8 NeuronCores.
#
# Math summary (per sample):
#   feat_up = A^T @ F @ A  (A = exact 160->640 bilinear weight matrix, fp16)
#   heatmap = last-writer-wins paint of 128 axis-aligned rects (value z_n)
#   loss    = mean((feat_up - heatmap)^2) over all pixels
#
# Rasterization: 2 paint matmuls per row-tile over box indicators with
# per-box weights w_n = 2^(n-65) (exponent encode, single group):
#   C  = sum_n w_n [covered] + eps      CA = sum_n (-z_n) w_n [covered]
# Per-pixel decode (bf16, exact when coverage depth <= 1, ~exact depth 2):
#   E2 = 2^(top exponent + 1) via int16 bit trick on C
#   den = E2 - C  (Sterbenz-exact);  Zneg = clamp(CA / den, -2, 2) = -z_top
# The -z map is then accumulated into the feat PSUM tile via an identity
# matmul, so (feat_up - z) forms in PSUM and the Act engine squares +
# accumulates it into the loss in one op.

import os
import numpy as np

B, C_IN, H, W = 32, 1, 160, 160
UP = 4
HO, WO = H * UP, W * UP
NBOX = 128
NCORES = 8
SPC = B // NCORES  # samples per core
NPIX = float(B * HO * WO)

_CACHE = {}


def _resize_matrix():
    """Exact bilinear (half-pixel centers, edge-clamped) 160->640 matrix,
    matching jax.image.resize(method='bilinear') for upsampling.
    All entries are multiples of 1/8 -> exact in fp16."""
    n_in, n_out = H, HO
    scale = n_out / n_in
    x = (np.arange(n_out, dtype=np.float64) + 0.5) / scale - 0.5
    k = np.arange(n_in, dtype=np.float64)
    w = np.maximum(0.0, 1.0 - np.abs(x[None, :] - k[:, None]))  # [in, out]
    w = w / w.sum(axis=0, keepdims=True)
    return w.astype(np.float16)


def _build(krep=1):
    import concourse.bacc as bacc
    import concourse.mybir as mybir
    from concourse.tile import TileContext

    fp32 = mybir.dt.float32
    bf16 = mybir.dt.bfloat16
    fp16 = mybir.dt.float16
    i16 = mybir.dt.int16
    u16 = mybir.dt.uint16
    i32 = mybir.dt.int32
    Alu = mybir.AluOpType
    ActF = mybir.ActivationFunctionType

    nc = bacc.Bacc("TRN2", target_bir_lowering=False, debug=False,
                   enable_asserts=False, num_devices=NCORES)
    feat_d = nc.dram_tensor("feat", [SPC, H, W], fp16, kind="ExternalInput")
    box_d = nc.dram_tensor("boxes", [SPC, NBOX, 5], fp32, kind="ExternalInput")
    amat_d = nc.dram_tensor("amat", [H, HO], fp16, kind="ExternalInput")
    iden_d = nc.dram_tensor("iden", [128, 128], fp32, kind="ExternalInput")
    out_d = nc.dram_tensor("out", [1, 1], fp32, kind="ExternalOutput")
    debug = os.environ.get("KV_DEBUG", "0") == "1"
    dbg_d = {}
    if debug:
        for nm in ("zc", "tf", "u", "v0",
                   "vs", "va", "ts", "ta", "tx", "ty"):
            dbg_d[nm] = nc.dram_tensor(f"dbg_{nm}", [128, HO], fp32,
                                       kind="ExternalOutput")

    EPS = float(2.0 ** -94)
    BANKS = (slice(0, 512), slice(512, 640))

    with TileContext(nc, num_cores=NCORES) as tc:
        with tc.tile_pool(name="const", bufs=1) as cpool, \
             tc.tile_pool(name="samp", bufs=2) as spool, \
             tc.tile_pool(name="dec", bufs=2) as dpool, \
             tc.tile_pool(name="psS", bufs=1, space="PSUM") as psS, \
             tc.tile_pool(name="psA", bufs=2, space="PSUM") as psA, \
             tc.tile_pool(name="psF", bufs=1, space="PSUM") as psF:

            # ---- constants ----
            A0h = cpool.tile([128, HO], fp16, tag="A0h")
            A1h = cpool.tile([32, HO], fp16, tag="A1h")
            nc.sync.dma_start(A0h[:], amat_d.ap()[0:128, :])
            nc.sync.dma_start(A1h[:], amat_d.ap()[128:160, :])

            iden32 = cpool.tile([128, 128], fp32, tag="iden32")
            nc.sync.dma_start(iden32[:], iden_d.ap())
            idbf = cpool.tile([128, 128], bf16, tag="idbf")
            nc.vector.tensor_copy(idbf[:], iden32[:])

            iota_32 = cpool.tile([128, HO], i32, tag="iot32")
            nc.gpsimd.iota(iota_32[:], pattern=[[1, HO]], base=0,
                           channel_multiplier=0)
            iota16 = cpool.tile([128, HO], i16, tag="iot16")
            nc.vector.tensor_copy(iota16[:], iota_32[:])

            nidx_i = cpool.tile([128, 1], i32, tag="nidxi")
            nc.gpsimd.iota(nidx_i[:], pattern=[[1, 1]], base=1,
                           channel_multiplier=1)  # n' = n+1 in 1..128
            w2b = cpool.tile([128, 1], i32, tag="w2b")
            nc.vector.tensor_scalar(w2b[:], nidx_i[:], 125, None, Alu.add)
            nc.vector.tensor_scalar(w2b[:], w2b[:], 23, None,
                                    Alu.logical_shift_left)
            # w2n = 2^(n'-2), exact in bf16; view as fp32 for ptr use
            w2n = w2b[:].bitcast(fp32)

            ones_t = cpool.tile([128, 1], fp32, tag="ones")
            nc.vector.memset(ones_t[:], 1.0)

            accbuf = cpool.tile([128, krep * SPC * 5], fp32, tag="acc")

            for rep in range(krep):
                for s in range(SPC):
                    # ---- loads ----
                    F0 = spool.tile([128, W], fp16, tag="F0")
                    F1 = spool.tile([32, W], fp16, tag="F1")
                    nc.sync.dma_start(F0[:], feat_d.ap()[s, 0:128, :])
                    nc.sync.dma_start(F1[:], feat_d.ap()[s, 128:160, :])
                    bx = spool.tile([128, 5], fp32, tag="bx")
                    nc.sync.dma_start(bx[:], box_d.ap()[s])
                    xq = bx[:, 0:1]
                    yq = bx[:, 1:2]
                    zq = bx[:, 2:3]
                    wq = bx[:, 3:4]
                    lq = bx[:, 4:5]

                    # ---- box prep (all [128,1]) ----
                    # HW float->int convert rounds to nearest, so floor(v)
                    # is computed as convert(v - 0.5).
                    cxi = dpool.tile([128, 1], i16, tag="cxi")
                    nc.vector.tensor_scalar(cxi[:], xq, -0.5, None, Alu.add)
                    cyi = dpool.tile([128, 1], i16, tag="cyi")
                    nc.vector.tensor_scalar(cyi[:], yq, -0.5, None, Alu.add)
                    # h = max(floor(w/2), 3) = round(max(w*0.5 - 0.5, 2.6));
                    # 2.6 not 2.5: round-half-even(2.5) = 2 would break MIN_RADIUS
                    hwf = dpool.tile([128, 1], fp32, tag="hwf")
                    nc.vector.tensor_scalar(hwf[:], wq, 0.5, -0.5, Alu.mult,
                                            Alu.add)
                    hwi = dpool.tile([128, 1], i16, tag="hwi")
                    nc.vector.tensor_scalar(hwi[:], hwf[:], 2.6, None, Alu.max)
                    hlf = dpool.tile([128, 1], fp32, tag="hlf")
                    nc.vector.tensor_scalar(hlf[:], lq, 0.5, -0.5, Alu.mult,
                                            Alu.add)
                    hli = dpool.tile([128, 1], i16, tag="hli")
                    nc.vector.tensor_scalar(hli[:], hlf[:], 2.6, None, Alu.max)
                    xmini = dpool.tile([128, 1], i16, tag="xmini")
                    nc.vector.tensor_tensor(xmini[:], cxi[:], hwi[:],
                                            Alu.subtract)
                    ymini = dpool.tile([128, 1], i16, tag="ymini")
                    nc.vector.tensor_tensor(ymini[:], cyi[:], hli[:],
                                            Alu.subtract)
                    # len = xmax - xmin = 2h + 1
                    lenxi = dpool.tile([128, 1], i16, tag="lenxi")
                    nc.vector.tensor_scalar(lenxi[:], hwi[:], 2, 1, Alu.mult,
                                            Alu.add)
                    lenyi = dpool.tile([128, 1], i16, tag="lenyi")
                    nc.vector.tensor_scalar(lenyi[:], hli[:], 2, 1, Alu.mult,
                                            Alu.add)
                    # fp32 views of the per-box scalars (scalar-ptr operands
                    # must be fp32)
                    xminf = dpool.tile([128, 1], fp32, tag="xminf")
                    nc.vector.tensor_copy(xminf[:], xmini[:])
                    yminf = dpool.tile([128, 1], fp32, tag="yminf")
                    nc.vector.tensor_copy(yminf[:], ymini[:])
                    lenxf = dpool.tile([128, 1], fp32, tag="lenxf")
                    nc.vector.tensor_copy(lenxf[:], lenxi[:])
                    lenyf = dpool.tile([128, 1], fp32, tag="lenyf")
                    nc.vector.tensor_copy(lenyf[:], lenyi[:])
                    # validity and paint weights
                    vw = dpool.tile([128, 1], fp32, tag="vw")
                    nc.vector.tensor_scalar(vw[:], wq, 0.0, None, Alu.is_gt)
                    vv = dpool.tile([128, 1], fp32, tag="vv")
                    nc.vector.scalar_tensor_tensor(vv[:], lq, 0.0, vw[:],
                                                   Alu.is_gt, Alu.logical_and)
                    wsv = dpool.tile([128, 1], fp32, tag="wsv")
                    nc.vector.tensor_tensor(wsv[:], w2n, vv[:], Alu.mult)
                    wav = dpool.tile([128, 1], fp32, tag="wav")
                    nc.vector.scalar_tensor_tensor(wav[:], zq, -1.0, wsv[:],
                                                   Alu.mult, Alu.mult)

                    # ---- U (row indicator) / V (col) in bf16 via int16 ----
                    tx = spool.tile([128, HO], i16, tag="tx")
                    nc.vector.tensor_scalar(tx[:], iota16[:], xminf[:], None,
                                            Alu.subtract)
                    U = spool.tile([128, HO], bf16, tag="U")
                    nc.vector.tensor_scalar(U[:], tx[:].bitcast(u16),
                                            lenxf[:], None, Alu.is_lt)
                    ty = spool.tile([128, HO], i16, tag="ty")
                    nc.vector.tensor_scalar(ty[:], iota16[:], yminf[:], None,
                                            Alu.subtract)
                    V0 = spool.tile([128, HO], bf16, tag="V0")
                    nc.vector.tensor_scalar(V0[:], ty[:].bitcast(u16),
                                            lenyf[:], None, Alu.is_lt)
                    Vs = spool.tile([128, HO], bf16, tag="Vs")
                    nc.vector.tensor_scalar(Vs[:], V0[:], wsv[:], None,
                                            Alu.mult)
                    Va = spool.tile([128, HO], bf16, tag="Va")
                    nc.vector.tensor_scalar(Va[:], V0[:], wav[:], None,
                                            Alu.mult)

                    # ---- step1: out1 = F^T A (row resize), fp16 ----
                    out1a = spool.tile([128, HO], fp16, tag="out1a")
                    out1b = spool.tile([32, HO], fp16, tag="out1b")
                    for part, (msz, moff, o1) in enumerate(
                            [(128, 0, out1a), (32, 128, out1b)]):
                        p1 = psS.tile([128, HO], fp32, tag="Ts")
                        for hs in BANKS:
                            nc.tensor.matmul(p1[0:msz, hs],
                                             F0[:, moff:moff + msz],
                                             A0h[:, hs], start=True,
                                             stop=False)
                            nc.tensor.matmul(p1[0:msz, hs],
                                             F1[:, moff:moff + msz],
                                             A1h[:, hs], start=False,
                                             stop=True)
                        nc.scalar.copy(o1[:], p1[0:msz, :])

                    # ---- per row-tile: paints + decode + loss ----
                    for m in range(5):
                        ms = slice(m * 128, (m + 1) * 128)
                        idx = ((rep * SPC + s) * 5) + m

                        Ts = psS.tile([128, HO], fp32, tag="Ts")
                        Ta = psA.tile([128, HO], fp32, tag="Ta")
                        Tf = psF.tile([128, HO], fp32, tag="Tf")
                        for hs in BANKS:
                            nc.tensor.matmul(Ts[:, hs], U[:, ms], Vs[:, hs],
                                             start=True, stop=True)
                            nc.tensor.matmul(Ta[:, hs], U[:, ms], Va[:, hs],
                                             start=True, stop=True)

                        # decode: Z = CA / 2^(top exponent of C), done as
                        # integer exponent subtraction on the raw fp32 bits.
                        # E1b = max(exp-bits(C) - bias, 0): uncovered pixels
                        # (C = 0) then decode to exactly 0.
                        E1i = dpool.tile([128, HO], i32, tag="E1i")
                        nc.vector.tensor_scalar(E1i[:], Ts[:].bitcast(i32),
                                                2139095040, None,
                                                Alu.bitwise_and)
                        E1b = dpool.tile([128, HO], i32, tag="E1b")
                        nc.gpsimd.tensor_scalar(E1b[:], E1i[:], 1065353216,
                                                0, Alu.subtract, Alu.max)
                        Zb = dpool.tile([128, HO], i32, tag="Zb")
                        nc.vector.tensor_tensor(Zb[:], Ta[:].bitcast(i32),
                                                E1b[:], Alu.subtract)
                        Zc = dpool.tile([128, HO], bf16, tag="Zc")
                        nc.gpsimd.tensor_scalar(Zc[:], Zb[:].bitcast(fp32),
                                                -2.0, 2.0, Alu.max, Alu.min)

                        # feat resize + (-z) paint: Tf := feat_up - z.
                        # Each bank's accumulation group is contiguous.
                        for hs in BANKS:
                            nc.tensor.matmul(Tf[:, hs], out1a[:, ms],
                                             A0h[:, hs], start=True,
                                             stop=False)
                            nc.tensor.matmul(Tf[:, hs], out1b[:, ms],
                                             A1h[:, hs], start=False,
                                             stop=False)
                            nc.tensor.matmul(Tf[:, hs], idbf[:], Zc[:, hs],
                                             start=False, stop=True)

                        # loss: square + accumulate straight from PSUM
                        dsq = dpool.tile([128, HO], bf16, tag="dsq")
                        nc.scalar.activation(
                            dsq[:], Tf[:], ActF.Square,
                            accum_out=accbuf[:, idx:idx + 1])

                        if debug and rep == 0 and s == 0 and m == 0:
                            def dump(nm, ap):
                                t = cpool.tile([128, HO], fp32, tag=f"dbg{nm}")
                                nc.vector.tensor_copy(t[:], ap)
                                nc.sync.dma_start(dbg_d[nm].ap(), t[:])
                            dump("zc", Zc[:])
                            dump("tf", Tf[:])
                            dump("u", U[:])
                            dump("v0", V0[:])
                            dump("vs", Vs[:])
                            dump("va", Va[:])
                            dump("ts", Ts[:])
                            dump("ta", Ta[:])
                            dump("tx", tx[:])
                            dump("ty", ty[:])

            # ---- final reduction ----
            tot = cpool.tile([128, 1], fp32, tag="tot")
            nc.vector.tensor_reduce(
                tot[:], accbuf[:, 0:krep * SPC * 5],
                mybir.AxisListType.X, Alu.add)
            if krep > 1:
                nc.vector.tensor_scalar(tot[:], tot[:], 1.0 / krep, None,
                                        Alu.mult)
            pfin = psA.tile([128, HO], fp32, tag="Ta")
            nc.tensor.matmul(pfin[0:1, 0:1], tot[:], ones_t[:],
                             start=True, stop=True)
            res = cpool.tile([1, 1], fp32, tag="res")
            nc.scalar.copy(res[:], pfin[0:1, 0:1])
            nc.sync.dma_start(out_d.ap(), res[:])

    nc.compile()
    return nc


def _get_nc(krep=1):
    key = ("nc", krep)
    if key not in _CACHE:
        _CACHE[key] = _build(krep)
    return _CACHE[key]


def run_cores(feat, gt_bboxes, krep=1):
    """Run the SPMD kernel; returns list of per-core sum-of-squared-diffs."""
    from concourse.bass_utils import run_bass_kernel_spmd
    nc = _get_nc(krep)
    amat = _resize_matrix()
    iden = np.eye(128, dtype=np.float32)
    feat = np.asarray(feat, dtype=np.float32)
    gt = np.ascontiguousarray(np.asarray(gt_bboxes, dtype=np.float32))
    feat16 = feat.astype(np.float16)
    in_maps = []
    for i in range(NCORES):
        sl = slice(i * SPC, (i + 1) * SPC)
        in_maps.append({
            "feat": np.ascontiguousarray(feat16[sl, 0]),
            "boxes": np.ascontiguousarray(gt[sl]),
            "amat": amat,
            "iden": iden,
        })
    res = run_bass_kernel_spmd(nc, in_maps, core_ids=list(range(NCORES)))
    return [float(res.results[i]["out"][0, 0]) for i in range(NCORES)]


def kernel(feat, gt_bboxes):
    parts = run_cores(feat, gt_bboxes, krep=1)
    total = float(np.sum(np.asarray(parts, dtype=np.float64)))
    return np.asarray(np.float32(total / NPIX))


# revision 21
# speedup vs baseline: 1.0342x; 1.0016x over previous
# Trainium2 Bass kernel for nn_CFTAuxHead (bilinear 4x resize + bbox
# rasterization + MSE loss), data-parallel over batch across 8 NeuronCores.
#
# Math summary (per sample):
#   feat_up = A^T @ F @ A  (A = exact 160->640 bilinear weight matrix, fp16)
#   heatmap = last-writer-wins paint of 128 axis-aligned rects (value z_n)
#   loss    = mean((feat_up - heatmap)^2) over all pixels
#
# Rasterization: 2 paint matmuls per row-tile over box indicators with
# per-box weights w_n = 2^(n-65) (exponent encode, single group):
#   C  = sum_n w_n [covered] + eps      CA = sum_n (-z_n) w_n [covered]
# Per-pixel decode (bf16, exact when coverage depth <= 1, ~exact depth 2):
#   E2 = 2^(top exponent + 1) via int16 bit trick on C
#   den = E2 - C  (Sterbenz-exact);  Zneg = clamp(CA / den, -2, 2) = -z_top
# The -z map is then accumulated into the feat PSUM tile via an identity
# matmul, so (feat_up - z) forms in PSUM and the Act engine squares +
# accumulates it into the loss in one op.

import os
import numpy as np

B, C_IN, H, W = 32, 1, 160, 160
UP = 4
HO, WO = H * UP, W * UP
NBOX = 128
NCORES = 8
SPC = B // NCORES  # samples per core
NPIX = float(B * HO * WO)

_CACHE = {}


def _resize_matrix():
    """Exact bilinear (half-pixel centers, edge-clamped) 160->640 matrix,
    matching jax.image.resize(method='bilinear') for upsampling.
    All entries are multiples of 1/8 -> exact in fp16."""
    n_in, n_out = H, HO
    scale = n_out / n_in
    x = (np.arange(n_out, dtype=np.float64) + 0.5) / scale - 0.5
    k = np.arange(n_in, dtype=np.float64)
    w = np.maximum(0.0, 1.0 - np.abs(x[None, :] - k[:, None]))  # [in, out]
    w = w / w.sum(axis=0, keepdims=True)
    return w.astype(np.float16)


def _build(krep=1):
    import concourse.bacc as bacc
    import concourse.mybir as mybir
    from concourse.tile import TileContext

    fp32 = mybir.dt.float32
    bf16 = mybir.dt.bfloat16
    fp16 = mybir.dt.float16
    i16 = mybir.dt.int16
    u16 = mybir.dt.uint16
    i32 = mybir.dt.int32
    Alu = mybir.AluOpType
    ActF = mybir.ActivationFunctionType

    nc = bacc.Bacc("TRN2", target_bir_lowering=False, debug=False,
                   enable_asserts=False, num_devices=NCORES)
    feat_d = nc.dram_tensor("feat", [SPC, H, W], fp16, kind="ExternalInput")
    box_d = nc.dram_tensor("boxes", [SPC, NBOX, 5], fp32, kind="ExternalInput")
    amat_d = nc.dram_tensor("amat", [H, HO], fp16, kind="ExternalInput")
    iden_d = nc.dram_tensor("iden", [128, 128], fp32, kind="ExternalInput")
    out_d = nc.dram_tensor("out", [1, 1], fp32, kind="ExternalOutput")
    debug = os.environ.get("KV_DEBUG", "0") == "1"
    dbg_d = {}
    if debug:
        for nm in ("zc", "tf", "u", "v0",
                   "vs", "va", "ts", "ta", "tx", "ty"):
            dbg_d[nm] = nc.dram_tensor(f"dbg_{nm}", [128, HO], fp32,
                                       kind="ExternalOutput")

    EPS = float(2.0 ** -94)
    BANKS = (slice(0, 512), slice(512, 640))

    with TileContext(nc, num_cores=NCORES) as tc:
        with tc.tile_pool(name="const", bufs=1) as cpool, \
             tc.tile_pool(name="samp", bufs=2) as spool, \
             tc.tile_pool(name="dec", bufs=4) as dpool, \
             tc.tile_pool(name="psS", bufs=1, space="PSUM") as psS, \
             tc.tile_pool(name="psA", bufs=2, space="PSUM") as psA, \
             tc.tile_pool(name="psF", bufs=1, space="PSUM") as psF:

            # ---- constants ----
            A0h = cpool.tile([128, HO], fp16, tag="A0h")
            A1h = cpool.tile([32, HO], fp16, tag="A1h")
            nc.sync.dma_start(A0h[:], amat_d.ap()[0:128, :])
            nc.sync.dma_start(A1h[:], amat_d.ap()[128:160, :])

            iden32 = cpool.tile([128, 128], fp32, tag="iden32")
            nc.sync.dma_start(iden32[:], iden_d.ap())
            idbf = cpool.tile([128, 128], bf16, tag="idbf")
            nc.vector.tensor_copy(idbf[:], iden32[:])

            iota_32 = cpool.tile([128, HO], i32, tag="iot32")
            nc.gpsimd.iota(iota_32[:], pattern=[[1, HO]], base=0,
                           channel_multiplier=0)
            iota16 = cpool.tile([128, HO], i16, tag="iot16")
            nc.vector.tensor_copy(iota16[:], iota_32[:])

            nidx_i = cpool.tile([128, 1], i32, tag="nidxi")
            nc.gpsimd.iota(nidx_i[:], pattern=[[1, 1]], base=1,
                           channel_multiplier=1)  # n' = n+1 in 1..128
            w2b = cpool.tile([128, 1], i32, tag="w2b")
            nc.vector.tensor_scalar(w2b[:], nidx_i[:], 125, None, Alu.add)
            nc.vector.tensor_scalar(w2b[:], w2b[:], 23, None,
                                    Alu.logical_shift_left)
            # w2n = 2^(n'-2), exact in bf16; view as fp32 for ptr use
            w2n = w2b[:].bitcast(fp32)

            ones_t = cpool.tile([128, 1], fp32, tag="ones")
            nc.vector.memset(ones_t[:], 1.0)

            accbuf = cpool.tile([128, krep * SPC * 5], fp32, tag="acc")

            for rep in range(krep):
                for s in range(SPC):
                    # ---- loads ----
                    F0 = spool.tile([128, W], fp16, tag="F0")
                    F1 = spool.tile([32, W], fp16, tag="F1")
                    nc.sync.dma_start(F0[:], feat_d.ap()[s, 0:128, :])
                    nc.sync.dma_start(F1[:], feat_d.ap()[s, 128:160, :])
                    bx = spool.tile([128, 5], fp32, tag="bx")
                    nc.sync.dma_start(bx[:], box_d.ap()[s])
                    xq = bx[:, 0:1]
                    yq = bx[:, 1:2]
                    zq = bx[:, 2:3]
                    wq = bx[:, 3:4]
                    lq = bx[:, 4:5]

                    # ---- box prep (all [128,1]) ----
                    # HW float->int convert rounds to nearest, so floor(v)
                    # is computed as convert(v - 0.5).
                    cxi = dpool.tile([128, 1], i16, tag="cxi")
                    nc.vector.tensor_scalar(cxi[:], xq, -0.5, None, Alu.add)
                    cyi = dpool.tile([128, 1], i16, tag="cyi")
                    nc.vector.tensor_scalar(cyi[:], yq, -0.5, None, Alu.add)
                    # h = max(floor(w/2), 3) = round(max(w*0.5 - 0.5, 2.6));
                    # 2.6 not 2.5: round-half-even(2.5) = 2 would break MIN_RADIUS
                    hwf = dpool.tile([128, 1], fp32, tag="hwf")
                    nc.vector.tensor_scalar(hwf[:], wq, 0.5, -0.5, Alu.mult,
                                            Alu.add)
                    hwi = dpool.tile([128, 1], i16, tag="hwi")
                    nc.vector.tensor_scalar(hwi[:], hwf[:], 2.6, None, Alu.max)
                    hlf = dpool.tile([128, 1], fp32, tag="hlf")
                    nc.vector.tensor_scalar(hlf[:], lq, 0.5, -0.5, Alu.mult,
                                            Alu.add)
                    hli = dpool.tile([128, 1], i16, tag="hli")
                    nc.vector.tensor_scalar(hli[:], hlf[:], 2.6, None, Alu.max)
                    xmini = dpool.tile([128, 1], i16, tag="xmini")
                    nc.vector.tensor_tensor(xmini[:], cxi[:], hwi[:],
                                            Alu.subtract)
                    ymini = dpool.tile([128, 1], i16, tag="ymini")
                    nc.vector.tensor_tensor(ymini[:], cyi[:], hli[:],
                                            Alu.subtract)
                    # len = xmax - xmin = 2h + 1
                    lenxi = dpool.tile([128, 1], i16, tag="lenxi")
                    nc.vector.tensor_scalar(lenxi[:], hwi[:], 2, 1, Alu.mult,
                                            Alu.add)
                    lenyi = dpool.tile([128, 1], i16, tag="lenyi")
                    nc.vector.tensor_scalar(lenyi[:], hli[:], 2, 1, Alu.mult,
                                            Alu.add)
                    # fp32 views of the per-box scalars (scalar-ptr operands
                    # must be fp32)
                    xminf = dpool.tile([128, 1], fp32, tag="xminf")
                    nc.vector.tensor_copy(xminf[:], xmini[:])
                    yminf = dpool.tile([128, 1], fp32, tag="yminf")
                    nc.vector.tensor_copy(yminf[:], ymini[:])
                    lenxf = dpool.tile([128, 1], fp32, tag="lenxf")
                    nc.vector.tensor_copy(lenxf[:], lenxi[:])
                    lenyf = dpool.tile([128, 1], fp32, tag="lenyf")
                    nc.vector.tensor_copy(lenyf[:], lenyi[:])
                    # validity and paint weights
                    vw = dpool.tile([128, 1], fp32, tag="vw")
                    nc.vector.tensor_scalar(vw[:], wq, 0.0, None, Alu.is_gt)
                    vv = dpool.tile([128, 1], fp32, tag="vv")
                    nc.vector.scalar_tensor_tensor(vv[:], lq, 0.0, vw[:],
                                                   Alu.is_gt, Alu.logical_and)
                    wsv = dpool.tile([128, 1], fp32, tag="wsv")
                    nc.vector.tensor_tensor(wsv[:], w2n, vv[:], Alu.mult)
                    wav = dpool.tile([128, 1], fp32, tag="wav")
                    nc.vector.scalar_tensor_tensor(wav[:], zq, -1.0, wsv[:],
                                                   Alu.mult, Alu.mult)

                    # ---- U (row indicator) / V (col) in bf16 via int16 ----
                    tx = spool.tile([128, HO], i16, tag="tx")
                    nc.vector.tensor_scalar(tx[:], iota16[:], xminf[:], None,
                                            Alu.subtract)
                    U = spool.tile([128, HO], bf16, tag="U")
                    nc.vector.tensor_scalar(U[:], tx[:].bitcast(u16),
                                            lenxf[:], None, Alu.is_lt)
                    ty = spool.tile([128, HO], i16, tag="ty")
                    nc.vector.tensor_scalar(ty[:], iota16[:], yminf[:], None,
                                            Alu.subtract)
                    V0 = spool.tile([128, HO], bf16, tag="V0")
                    nc.vector.tensor_scalar(V0[:], ty[:].bitcast(u16),
                                            lenyf[:], None, Alu.is_lt)
                    Vs = spool.tile([128, HO], bf16, tag="Vs")
                    nc.vector.tensor_scalar(Vs[:], V0[:], wsv[:], None,
                                            Alu.mult)
                    Va = spool.tile([128, HO], bf16, tag="Va")
                    nc.vector.tensor_scalar(Va[:], V0[:], wav[:], None,
                                            Alu.mult)

                    # ---- step1: out1 = F^T A (row resize), fp16 ----
                    out1a = spool.tile([128, HO], fp16, tag="out1a")
                    out1b = spool.tile([32, HO], fp16, tag="out1b")
                    for part, (msz, moff, o1) in enumerate(
                            [(128, 0, out1a), (32, 128, out1b)]):
                        p1 = psS.tile([128, HO], fp32, tag="Ts")
                        for hs in BANKS:
                            nc.tensor.matmul(p1[0:msz, hs],
                                             F0[:, moff:moff + msz],
                                             A0h[:, hs], start=True,
                                             stop=False)
                            nc.tensor.matmul(p1[0:msz, hs],
                                             F1[:, moff:moff + msz],
                                             A1h[:, hs], start=False,
                                             stop=True)
                        nc.scalar.copy(o1[:], p1[0:msz, :])

                    # ---- per row-tile: paints + decode + loss ----
                    for m in range(5):
                        ms = slice(m * 128, (m + 1) * 128)
                        idx = ((rep * SPC + s) * 5) + m

                        Ts = psS.tile([128, HO], fp32, tag="Ts")
                        Ta = psA.tile([128, HO], fp32, tag="Ta")
                        Tf = psF.tile([128, HO], fp32, tag="Tf")
                        for hs in BANKS:
                            nc.tensor.matmul(Ts[:, hs], U[:, ms], Vs[:, hs],
                                             start=True, stop=True)
                            nc.tensor.matmul(Ta[:, hs], U[:, ms], Va[:, hs],
                                             start=True, stop=True)

                        # decode: Z = CA / 2^(top exponent of C), done as
                        # integer exponent subtraction on the raw fp32 bits.
                        # E1b = max(exp-bits(C) - bias, 0): uncovered pixels
                        # (C = 0) then decode to exactly 0.
                        E1i = dpool.tile([128, HO], i32, tag="E1i")
                        nc.vector.tensor_scalar(E1i[:], Ts[:].bitcast(i32),
                                                2139095040, None,
                                                Alu.bitwise_and)
                        E1b = dpool.tile([128, HO], i32, tag="E1b")
                        nc.gpsimd.tensor_scalar(E1b[:], E1i[:], 1065353216,
                                                0, Alu.subtract, Alu.max)
                        Zb = dpool.tile([128, HO], i32, tag="Zb")
                        nc.vector.tensor_tensor(Zb[:], Ta[:].bitcast(i32),
                                                E1b[:], Alu.subtract)
                        Zc = dpool.tile([128, HO], bf16, tag="Zc")
                        nc.gpsimd.tensor_scalar(Zc[:], Zb[:].bitcast(fp32),
                                                -2.0, 2.0, Alu.max, Alu.min)

                        # feat resize + (-z) paint: Tf := feat_up - z.
                        # Each bank's accumulation group is contiguous.
                        for hs in BANKS:
                            nc.tensor.matmul(Tf[:, hs], out1a[:, ms],
                                             A0h[:, hs], start=True,
                                             stop=False)
                            nc.tensor.matmul(Tf[:, hs], out1b[:, ms],
                                             A1h[:, hs], start=False,
                                             stop=False)
                            nc.tensor.matmul(Tf[:, hs], idbf[:], Zc[:, hs],
                                             start=False, stop=True)

                        # loss: square + accumulate straight from PSUM
                        dsq = dpool.tile([128, HO], bf16, tag="dsq")
                        nc.scalar.activation(
                            dsq[:], Tf[:], ActF.Square,
                            accum_out=accbuf[:, idx:idx + 1])

                        if debug and rep == 0 and s == 0 and m == 0:
                            def dump(nm, ap):
                                t = cpool.tile([128, HO], fp32, tag=f"dbg{nm}")
                                nc.vector.tensor_copy(t[:], ap)
                                nc.sync.dma_start(dbg_d[nm].ap(), t[:])
                            dump("zc", Zc[:])
                            dump("tf", Tf[:])
                            dump("u", U[:])
                            dump("v0", V0[:])
                            dump("vs", Vs[:])
                            dump("va", Va[:])
                            dump("ts", Ts[:])
                            dump("ta", Ta[:])
                            dump("tx", tx[:])
                            dump("ty", ty[:])

            # ---- final reduction ----
            tot = cpool.tile([128, 1], fp32, tag="tot")
            nc.vector.tensor_reduce(
                tot[:], accbuf[:, 0:krep * SPC * 5],
                mybir.AxisListType.X, Alu.add)
            if krep > 1:
                nc.vector.tensor_scalar(tot[:], tot[:], 1.0 / krep, None,
                                        Alu.mult)
            pfin = psA.tile([128, HO], fp32, tag="Ta")
            nc.tensor.matmul(pfin[0:1, 0:1], tot[:], ones_t[:],
                             start=True, stop=True)
            res = cpool.tile([1, 1], fp32, tag="res")
            nc.scalar.copy(res[:], pfin[0:1, 0:1])
            nc.sync.dma_start(out_d.ap(), res[:])

    nc.compile()
    return nc


def _get_nc(krep=1):
    key = ("nc", krep)
    if key not in _CACHE:
        _CACHE[key] = _build(krep)
    return _CACHE[key]


def run_cores(feat, gt_bboxes, krep=1):
    """Run the SPMD kernel; returns list of per-core sum-of-squared-diffs."""
    from concourse.bass_utils import run_bass_kernel_spmd
    nc = _get_nc(krep)
    amat = _resize_matrix()
    iden = np.eye(128, dtype=np.float32)
    feat = np.asarray(feat, dtype=np.float32)
    gt = np.ascontiguousarray(np.asarray(gt_bboxes, dtype=np.float32))
    feat16 = feat.astype(np.float16)
    in_maps = []
    for i in range(NCORES):
        sl = slice(i * SPC, (i + 1) * SPC)
        in_maps.append({
            "feat": np.ascontiguousarray(feat16[sl, 0]),
            "boxes": np.ascontiguousarray(gt[sl]),
            "amat": amat,
            "iden": iden,
        })
    res = run_bass_kernel_spmd(nc, in_maps, core_ids=list(range(NCORES)))
    return [float(res.results[i]["out"][0, 0]) for i in range(NCORES)]


def kernel(feat, gt_bboxes):
    parts = run_cores(feat, gt_bboxes, krep=1)
    total = float(np.sum(np.asarray(parts, dtype=np.float64)))
    return np.asarray(np.float32(total / NPIX))


# revision 22
# speedup vs baseline: 1.0533x; 1.0185x over previous
# Trainium2 Bass kernel for nn_CFTAuxHead (bilinear 4x resize + bbox
# rasterization + MSE loss), data-parallel over batch across 8 NeuronCores.
#
# Math summary (per sample):
#   feat_up = A^T @ F @ A  (A = exact 160->640 bilinear weight matrix, fp16)
#   heatmap = last-writer-wins paint of 128 axis-aligned rects (value z_n)
#   loss    = mean((feat_up - heatmap)^2) over all pixels
#
# Rasterization: 2 paint matmuls per row-tile over box indicators with
# per-box weights w_n = 2^(n-65) (exponent encode, single group):
#   C  = sum_n w_n [covered] + eps      CA = sum_n (-z_n) w_n [covered]
# Per-pixel decode (bf16, exact when coverage depth <= 1, ~exact depth 2):
#   E2 = 2^(top exponent + 1) via int16 bit trick on C
#   den = E2 - C  (Sterbenz-exact);  Zneg = clamp(CA / den, -2, 2) = -z_top
# The -z map is then accumulated into the feat PSUM tile via an identity
# matmul, so (feat_up - z) forms in PSUM and the Act engine squares +
# accumulates it into the loss in one op.

import os
import numpy as np

B, C_IN, H, W = 32, 1, 160, 160
UP = 4
HO, WO = H * UP, W * UP
NBOX = 128
NCORES = 8
SPC = B // NCORES  # samples per core
NPIX = float(B * HO * WO)

_CACHE = {}


def _resize_matrix():
    """Exact bilinear (half-pixel centers, edge-clamped) 160->640 matrix,
    matching jax.image.resize(method='bilinear') for upsampling.
    All entries are multiples of 1/8 -> exact in fp16."""
    n_in, n_out = H, HO
    scale = n_out / n_in
    x = (np.arange(n_out, dtype=np.float64) + 0.5) / scale - 0.5
    k = np.arange(n_in, dtype=np.float64)
    w = np.maximum(0.0, 1.0 - np.abs(x[None, :] - k[:, None]))  # [in, out]
    w = w / w.sum(axis=0, keepdims=True)
    return w.astype(np.float16)


def _build(krep=1):
    import concourse.bacc as bacc
    import concourse.mybir as mybir
    from concourse.tile import TileContext

    fp32 = mybir.dt.float32
    bf16 = mybir.dt.bfloat16
    fp16 = mybir.dt.float16
    i16 = mybir.dt.int16
    u16 = mybir.dt.uint16
    i32 = mybir.dt.int32
    Alu = mybir.AluOpType
    ActF = mybir.ActivationFunctionType

    nc = bacc.Bacc("TRN2", target_bir_lowering=False, debug=False,
                   enable_asserts=False, num_devices=NCORES)
    feat_d = nc.dram_tensor("feat", [SPC, H, W], fp16, kind="ExternalInput")
    box_d = nc.dram_tensor("boxes", [SPC, NBOX, 5], fp32, kind="ExternalInput")
    amat_d = nc.dram_tensor("amat", [H, HO], fp16, kind="ExternalInput")
    iden_d = nc.dram_tensor("iden", [128, 128], fp32, kind="ExternalInput")
    out_d = nc.dram_tensor("out", [1, 1], fp32, kind="ExternalOutput")
    debug = os.environ.get("KV_DEBUG", "0") == "1"
    dbg_d = {}
    if debug:
        for nm in ("zc", "tf", "u", "v0",
                   "vs", "va", "ts", "ta", "tx", "ty"):
            dbg_d[nm] = nc.dram_tensor(f"dbg_{nm}", [128, HO], fp32,
                                       kind="ExternalOutput")

    EPS = float(2.0 ** -94)
    BANKS = (slice(0, 512), slice(512, 640))

    with TileContext(nc, num_cores=NCORES) as tc:
        with tc.tile_pool(name="const", bufs=1) as cpool, \
             tc.tile_pool(name="samp", bufs=2) as spool, \
             tc.tile_pool(name="dec", bufs=4) as dpool, \
             tc.tile_pool(name="psS", bufs=1, space="PSUM") as psS, \
             tc.tile_pool(name="psA", bufs=1, space="PSUM") as psA, \
             tc.tile_pool(name="psF", bufs=2, space="PSUM") as psF:

            # ---- constants ----
            A0h = cpool.tile([128, HO], fp16, tag="A0h")
            A1h = cpool.tile([32, HO], fp16, tag="A1h")
            nc.sync.dma_start(A0h[:], amat_d.ap()[0:128, :])
            nc.sync.dma_start(A1h[:], amat_d.ap()[128:160, :])

            iden32 = cpool.tile([128, 128], fp32, tag="iden32")
            nc.sync.dma_start(iden32[:], iden_d.ap())
            idbf = cpool.tile([128, 128], bf16, tag="idbf")
            nc.vector.tensor_copy(idbf[:], iden32[:])

            iota_32 = cpool.tile([128, HO], i32, tag="iot32")
            nc.gpsimd.iota(iota_32[:], pattern=[[1, HO]], base=0,
                           channel_multiplier=0)
            iota16 = cpool.tile([128, HO], i16, tag="iot16")
            nc.vector.tensor_copy(iota16[:], iota_32[:])

            nidx_i = cpool.tile([128, 1], i32, tag="nidxi")
            nc.gpsimd.iota(nidx_i[:], pattern=[[1, 1]], base=1,
                           channel_multiplier=1)  # n' = n+1 in 1..128
            w2b = cpool.tile([128, 1], i32, tag="w2b")
            nc.vector.tensor_scalar(w2b[:], nidx_i[:], 125, None, Alu.add)
            nc.vector.tensor_scalar(w2b[:], w2b[:], 23, None,
                                    Alu.logical_shift_left)
            # w2n = 2^(n'-2), exact in bf16; view as fp32 for ptr use
            w2n = w2b[:].bitcast(fp32)

            ones_t = cpool.tile([128, 1], fp32, tag="ones")
            nc.vector.memset(ones_t[:], 1.0)

            accbuf = cpool.tile([128, krep * SPC * 5], fp32, tag="acc")
            pending = None

            for rep in range(krep):
                for s in range(SPC):
                    # ---- loads ----
                    F0 = spool.tile([128, W], fp16, tag="F0")
                    F1 = spool.tile([32, W], fp16, tag="F1")
                    nc.sync.dma_start(F0[:], feat_d.ap()[s, 0:128, :])
                    nc.sync.dma_start(F1[:], feat_d.ap()[s, 128:160, :])
                    bx = spool.tile([128, 5], fp32, tag="bx")
                    nc.sync.dma_start(bx[:], box_d.ap()[s])
                    xq = bx[:, 0:1]
                    yq = bx[:, 1:2]
                    zq = bx[:, 2:3]
                    wq = bx[:, 3:4]
                    lq = bx[:, 4:5]

                    # ---- box prep (all [128,1]) ----
                    # HW float->int convert rounds to nearest, so floor(v)
                    # is computed as convert(v - 0.5).
                    cxi = dpool.tile([128, 1], i16, tag="cxi")
                    nc.vector.tensor_scalar(cxi[:], xq, -0.5, None, Alu.add)
                    cyi = dpool.tile([128, 1], i16, tag="cyi")
                    nc.vector.tensor_scalar(cyi[:], yq, -0.5, None, Alu.add)
                    # h = max(floor(w/2), 3) = round(max(w*0.5 - 0.5, 2.6));
                    # 2.6 not 2.5: round-half-even(2.5) = 2 would break MIN_RADIUS
                    hwf = dpool.tile([128, 1], fp32, tag="hwf")
                    nc.vector.tensor_scalar(hwf[:], wq, 0.5, -0.5, Alu.mult,
                                            Alu.add)
                    hwi = dpool.tile([128, 1], i16, tag="hwi")
                    nc.vector.tensor_scalar(hwi[:], hwf[:], 2.6, None, Alu.max)
                    hlf = dpool.tile([128, 1], fp32, tag="hlf")
                    nc.vector.tensor_scalar(hlf[:], lq, 0.5, -0.5, Alu.mult,
                                            Alu.add)
                    hli = dpool.tile([128, 1], i16, tag="hli")
                    nc.vector.tensor_scalar(hli[:], hlf[:], 2.6, None, Alu.max)
                    xmini = dpool.tile([128, 1], i16, tag="xmini")
                    nc.vector.tensor_tensor(xmini[:], cxi[:], hwi[:],
                                            Alu.subtract)
                    ymini = dpool.tile([128, 1], i16, tag="ymini")
                    nc.vector.tensor_tensor(ymini[:], cyi[:], hli[:],
                                            Alu.subtract)
                    # len = xmax - xmin = 2h + 1
                    lenxi = dpool.tile([128, 1], i16, tag="lenxi")
                    nc.vector.tensor_scalar(lenxi[:], hwi[:], 2, 1, Alu.mult,
                                            Alu.add)
                    lenyi = dpool.tile([128, 1], i16, tag="lenyi")
                    nc.vector.tensor_scalar(lenyi[:], hli[:], 2, 1, Alu.mult,
                                            Alu.add)
                    # fp32 views of the per-box scalars (scalar-ptr operands
                    # must be fp32)
                    xminf = dpool.tile([128, 1], fp32, tag="xminf")
                    nc.vector.tensor_copy(xminf[:], xmini[:])
                    yminf = dpool.tile([128, 1], fp32, tag="yminf")
                    nc.vector.tensor_copy(yminf[:], ymini[:])
                    lenxf = dpool.tile([128, 1], fp32, tag="lenxf")
                    nc.vector.tensor_copy(lenxf[:], lenxi[:])
                    lenyf = dpool.tile([128, 1], fp32, tag="lenyf")
                    nc.vector.tensor_copy(lenyf[:], lenyi[:])
                    # validity and paint weights
                    vw = dpool.tile([128, 1], fp32, tag="vw")
                    nc.vector.tensor_scalar(vw[:], wq, 0.0, None, Alu.is_gt)
                    vv = dpool.tile([128, 1], fp32, tag="vv")
                    nc.vector.scalar_tensor_tensor(vv[:], lq, 0.0, vw[:],
                                                   Alu.is_gt, Alu.logical_and)
                    wsv = dpool.tile([128, 1], fp32, tag="wsv")
                    nc.vector.tensor_tensor(wsv[:], w2n, vv[:], Alu.mult)
                    wav = dpool.tile([128, 1], fp32, tag="wav")
                    nc.vector.scalar_tensor_tensor(wav[:], zq, -1.0, wsv[:],
                                                   Alu.mult, Alu.mult)

                    # ---- U (row indicator) / V (col) in bf16 via int16 ----
                    tx = spool.tile([128, HO], i16, tag="tx")
                    nc.vector.tensor_scalar(tx[:], iota16[:], xminf[:], None,
                                            Alu.subtract)
                    U = spool.tile([128, HO], bf16, tag="U")
                    nc.vector.tensor_scalar(U[:], tx[:].bitcast(u16),
                                            lenxf[:], None, Alu.is_lt)
                    ty = spool.tile([128, HO], i16, tag="ty")
                    nc.vector.tensor_scalar(ty[:], iota16[:], yminf[:], None,
                                            Alu.subtract)
                    V0 = spool.tile([128, HO], bf16, tag="V0")
                    nc.vector.tensor_scalar(V0[:], ty[:].bitcast(u16),
                                            lenyf[:], None, Alu.is_lt)
                    Vs = spool.tile([128, HO], bf16, tag="Vs")
                    nc.vector.tensor_scalar(Vs[:], V0[:], wsv[:], None,
                                            Alu.mult)
                    Va = spool.tile([128, HO], bf16, tag="Va")
                    nc.vector.tensor_scalar(Va[:], V0[:], wav[:], None,
                                            Alu.mult)

                    # ---- step1: out1 = F^T A (row resize), fp16 ----
                    out1a = spool.tile([128, HO], fp16, tag="out1a")
                    out1b = spool.tile([32, HO], fp16, tag="out1b")
                    for part, (msz, moff, o1) in enumerate(
                            [(128, 0, out1a), (32, 128, out1b)]):
                        p1 = psS.tile([128, HO], fp32, tag="Ts")
                        for hs in BANKS:
                            nc.tensor.matmul(p1[0:msz, hs],
                                             F0[:, moff:moff + msz],
                                             A0h[:, hs], start=True,
                                             stop=False)
                            nc.tensor.matmul(p1[0:msz, hs],
                                             F1[:, moff:moff + msz],
                                             A1h[:, hs], start=False,
                                             stop=True)
                        nc.scalar.copy(o1[:], p1[0:msz, :])

                    # ---- per row-tile: paints + decode + loss ----
                    for m in range(5):
                        ms = slice(m * 128, (m + 1) * 128)
                        idx = ((rep * SPC + s) * 5) + m

                        Ts = psS.tile([128, HO], fp32, tag="Ts")
                        Ta = psA.tile([128, HO], fp32, tag="Ta")
                        Tf = psF.tile([128, HO], fp32, tag="Tf")
                        for hs in BANKS:
                            nc.tensor.matmul(Ts[:, hs], U[:, ms], Vs[:, hs],
                                             start=True, stop=True)
                            nc.tensor.matmul(Ta[:, hs], U[:, ms], Va[:, hs],
                                             start=True, stop=True)

                        # deferred -z paint + loss for the previous tile:
                        # keeps the PE in-order queue from stalling on the
                        # current tile's decode chain.
                        if pending is not None:
                            pTf, pZc, pidx = pending
                            for hs in BANKS:
                                nc.tensor.matmul(pTf[:, hs], idbf[:],
                                                 pZc[:, hs], start=False,
                                                 stop=True)
                            pdsq = dpool.tile([128, HO], bf16, tag="dsq")
                            nc.scalar.activation(
                                pdsq[:], pTf[:], ActF.Square,
                                accum_out=accbuf[:, pidx:pidx + 1])
                            pending = None

                        # decode: Z = CA / 2^(top exponent of C), done as
                        # integer exponent subtraction on the raw fp32 bits.
                        # E1b = max(exp-bits(C) - bias, 0): uncovered pixels
                        # (C = 0) then decode to exactly 0.
                        E1i = dpool.tile([128, HO], i32, tag="E1i")
                        nc.vector.tensor_scalar(E1i[:], Ts[:].bitcast(i32),
                                                2139095040, None,
                                                Alu.bitwise_and)
                        E1b = dpool.tile([128, HO], i32, tag="E1b")
                        nc.gpsimd.tensor_scalar(E1b[:], E1i[:], 1065353216,
                                                0, Alu.subtract, Alu.max)
                        Zb = dpool.tile([128, HO], i32, tag="Zb")
                        nc.vector.tensor_tensor(Zb[:], Ta[:].bitcast(i32),
                                                E1b[:], Alu.subtract)
                        Zc = dpool.tile([128, HO], bf16, tag="Zc")
                        nc.gpsimd.tensor_scalar(Zc[:], Zb[:].bitcast(fp32),
                                                -2.0, 2.0, Alu.max, Alu.min)

                        # feat resize into Tf; the -z paint and the loss
                        # accumulation are deferred to the next tile.
                        for hs in BANKS:
                            nc.tensor.matmul(Tf[:, hs], out1a[:, ms],
                                             A0h[:, hs], start=True,
                                             stop=False)
                            nc.tensor.matmul(Tf[:, hs], out1b[:, ms],
                                             A1h[:, hs], start=False,
                                             stop=False)
                        pending = (Tf, Zc, idx)

                        if debug and rep == 0 and s == 0 and m == 0:
                            def dump(nm, ap):
                                t = cpool.tile([128, HO], fp32, tag=f"dbg{nm}")
                                nc.vector.tensor_copy(t[:], ap)
                                nc.sync.dma_start(dbg_d[nm].ap(), t[:])
                            dump("zc", Zc[:])
                            dump("tf", Tf[:])
                            dump("u", U[:])
                            dump("v0", V0[:])
                            dump("vs", Vs[:])
                            dump("va", Va[:])
                            dump("ts", Ts[:])
                            dump("ta", Ta[:])
                            dump("tx", tx[:])
                            dump("ty", ty[:])

            if pending is not None:
                pTf, pZc, pidx = pending
                for hs in BANKS:
                    nc.tensor.matmul(pTf[:, hs], idbf[:], pZc[:, hs],
                                     start=False, stop=True)
                pdsq = dpool.tile([128, HO], bf16, tag="dsq")
                nc.scalar.activation(pdsq[:], pTf[:], ActF.Square,
                                     accum_out=accbuf[:, pidx:pidx + 1])
                pending = None

            # ---- final reduction ----
            tot = cpool.tile([128, 1], fp32, tag="tot")
            nc.vector.tensor_reduce(
                tot[:], accbuf[:, 0:krep * SPC * 5],
                mybir.AxisListType.X, Alu.add)
            if krep > 1:
                nc.vector.tensor_scalar(tot[:], tot[:], 1.0 / krep, None,
                                        Alu.mult)
            pfin = psA.tile([128, HO], fp32, tag="Ta")
            nc.tensor.matmul(pfin[0:1, 0:1], tot[:], ones_t[:],
                             start=True, stop=True)
            res = cpool.tile([1, 1], fp32, tag="res")
            nc.scalar.copy(res[:], pfin[0:1, 0:1])
            nc.sync.dma_start(out_d.ap(), res[:])

    nc.compile()
    return nc


def _get_nc(krep=1):
    key = ("nc", krep)
    if key not in _CACHE:
        _CACHE[key] = _build(krep)
    return _CACHE[key]


def run_cores(feat, gt_bboxes, krep=1):
    """Run the SPMD kernel; returns list of per-core sum-of-squared-diffs."""
    from concourse.bass_utils import run_bass_kernel_spmd
    nc = _get_nc(krep)
    amat = _resize_matrix()
    iden = np.eye(128, dtype=np.float32)
    feat = np.asarray(feat, dtype=np.float32)
    gt = np.ascontiguousarray(np.asarray(gt_bboxes, dtype=np.float32))
    feat16 = feat.astype(np.float16)
    in_maps = []
    for i in range(NCORES):
        sl = slice(i * SPC, (i + 1) * SPC)
        in_maps.append({
            "feat": np.ascontiguousarray(feat16[sl, 0]),
            "boxes": np.ascontiguousarray(gt[sl]),
            "amat": amat,
            "iden": iden,
        })
    res = run_bass_kernel_spmd(nc, in_maps, core_ids=list(range(NCORES)))
    return [float(res.results[i]["out"][0, 0]) for i in range(NCORES)]


def kernel(feat, gt_bboxes):
    parts = run_cores(feat, gt_bboxes, krep=1)
    total = float(np.sum(np.asarray(parts, dtype=np.float64)))
    return np.asarray(np.float32(total / NPIX))


# revision 24
# speedup vs baseline: 1.2088x; 1.1477x over previous
# Trainium2 Bass kernel for nn_CFTAuxHead (bilinear 4x resize + bbox
# rasterization + MSE loss), data-parallel over batch across 8 NeuronCores.
#
# Math summary (per sample):
#   feat_up = A^T @ F @ A  (A = exact 160->640 bilinear weight matrix, fp16)
#   heatmap = last-writer-wins paint of 128 axis-aligned rects (value z_n)
#   loss    = mean((feat_up - heatmap)^2) over all pixels
#
# Rasterization: 2 paint matmuls per row-tile over box indicators with
# per-box weights w_n = 2^(n-65) (exponent encode, single group):
#   C  = sum_n w_n [covered] + eps      CA = sum_n (-z_n) w_n [covered]
# Per-pixel decode (bf16, exact when coverage depth <= 1, ~exact depth 2):
#   E2 = 2^(top exponent + 1) via int16 bit trick on C
#   den = E2 - C  (Sterbenz-exact);  Zneg = clamp(CA / den, -2, 2) = -z_top
# The -z map is then accumulated into the feat PSUM tile via an identity
# matmul, so (feat_up - z) forms in PSUM and the Act engine squares +
# accumulates it into the loss in one op.

import os
import numpy as np

B, C_IN, H, W = 32, 1, 160, 160
UP = 4
HO, WO = H * UP, W * UP
NBOX = 128
NCORES = 8
SPC = B // NCORES  # samples per core
NPIX = float(B * HO * WO)

_CACHE = {}


def _resize_matrix():
    """Exact bilinear (half-pixel centers, edge-clamped) 160->640 matrix,
    matching jax.image.resize(method='bilinear') for upsampling.
    All entries are multiples of 1/8 -> exact in fp16."""
    n_in, n_out = H, HO
    scale = n_out / n_in
    x = (np.arange(n_out, dtype=np.float64) + 0.5) / scale - 0.5
    k = np.arange(n_in, dtype=np.float64)
    w = np.maximum(0.0, 1.0 - np.abs(x[None, :] - k[:, None]))  # [in, out]
    w = w / w.sum(axis=0, keepdims=True)
    return w.astype(np.float16)


def _build(krep=1):
    import concourse.bacc as bacc
    import concourse.mybir as mybir
    from concourse.tile import TileContext

    fp32 = mybir.dt.float32
    bf16 = mybir.dt.bfloat16
    fp16 = mybir.dt.float16
    i16 = mybir.dt.int16
    u16 = mybir.dt.uint16
    i32 = mybir.dt.int32
    Alu = mybir.AluOpType
    ActF = mybir.ActivationFunctionType

    nc = bacc.Bacc("TRN2", target_bir_lowering=False, debug=False,
                   enable_asserts=False, num_devices=NCORES)
    feat_d = nc.dram_tensor("feat", [SPC, H, W], fp16, kind="ExternalInput")
    box_d = nc.dram_tensor("boxes", [SPC, NBOX, 5], fp32, kind="ExternalInput")
    amat_d = nc.dram_tensor("amat", [H, HO], fp16, kind="ExternalInput")
    iden_d = nc.dram_tensor("iden", [128, 128], fp32, kind="ExternalInput")
    out_d = nc.dram_tensor("out", [1, 1], fp32, kind="ExternalOutput")

    EPS = float(2.0 ** -94)
    BANKS = (slice(0, 512), slice(512, 640))

    with TileContext(nc, num_cores=NCORES) as tc:
        with tc.tile_pool(name="const", bufs=1) as cpool, \
             tc.tile_pool(name="samp", bufs=2) as spool, \
             tc.tile_pool(name="dec", bufs=4) as dpool, \
             tc.tile_pool(name="psS", bufs=1, space="PSUM") as psS, \
             tc.tile_pool(name="psA", bufs=1, space="PSUM") as psA, \
             tc.tile_pool(name="psF", bufs=2, space="PSUM") as psF:

            # ---- constants ----
            A0h = cpool.tile([128, HO], fp16, tag="A0h")
            A1h = cpool.tile([32, HO], fp16, tag="A1h")
            nc.sync.dma_start(A0h[:], amat_d.ap()[0:128, :])
            nc.sync.dma_start(A1h[:], amat_d.ap()[128:160, :])

            iden32 = cpool.tile([128, 128], fp32, tag="iden32")
            nc.sync.dma_start(iden32[:], iden_d.ap())
            idbf = cpool.tile([128, 128], bf16, tag="idbf")
            nc.vector.tensor_copy(idbf[:], iden32[:])

            iota_32 = cpool.tile([128, HO], i32, tag="iot32")
            nc.gpsimd.iota(iota_32[:], pattern=[[1, HO]], base=0,
                           channel_multiplier=0)
            iota16 = cpool.tile([128, HO], i16, tag="iot16")
            nc.vector.tensor_copy(iota16[:], iota_32[:])

            nidx_i = cpool.tile([128, 1], i32, tag="nidxi")
            nc.gpsimd.iota(nidx_i[:], pattern=[[1, 1]], base=1,
                           channel_multiplier=1)  # n' = n+1 in 1..128
            w2b = cpool.tile([128, 1], i32, tag="w2b")
            nc.vector.tensor_scalar(w2b[:], nidx_i[:], 125, None, Alu.add)
            nc.vector.tensor_scalar(w2b[:], w2b[:], 23, None,
                                    Alu.logical_shift_left)
            # w2n = 2^(n'-2), exact in bf16; view as fp32 for ptr use
            w2n = w2b[:].bitcast(fp32)

            ones_t = cpool.tile([128, 1], fp32, tag="ones")
            nc.vector.memset(ones_t[:], 1.0)

            accbuf = cpool.tile([128, krep * SPC * 5], fp32, tag="acc")
            pending = None

            def prep(rep, s):
                """Loads + box prep + U/V build + row-resize for one sample.
                Emitted one sample ahead, interleaved into the previous
                sample's tile loop, so its DVE/PE work fills pipeline gaps."""
                F0 = spool.tile([128, W], fp16, tag="F0")
                F1 = spool.tile([32, W], fp16, tag="F1")
                nc.sync.dma_start(F0[:], feat_d.ap()[s, 0:128, :])
                nc.sync.dma_start(F1[:], feat_d.ap()[s, 128:160, :])
                bx = spool.tile([128, 5], fp32, tag="bx")
                nc.sync.dma_start(bx[:], box_d.ap()[s])
                xq = bx[:, 0:1]
                yq = bx[:, 1:2]
                zq = bx[:, 2:3]
                wq = bx[:, 3:4]
                lq = bx[:, 4:5]

                # HW float->int convert rounds to nearest, so floor(v) is
                # computed as convert(v - 0.5).
                cxi = dpool.tile([128, 1], i16, tag="cxi")
                nc.vector.tensor_scalar(cxi[:], xq, -0.5, None, Alu.add)
                cyi = dpool.tile([128, 1], i16, tag="cyi")
                nc.vector.tensor_scalar(cyi[:], yq, -0.5, None, Alu.add)
                # h = max(floor(w/2), 3) = round(max(w*0.5 - 0.5, 2.6));
                # 2.6 not 2.5: round-half-even(2.5) = 2 would break MIN_RADIUS
                hwf = dpool.tile([128, 1], fp32, tag="hwf")
                nc.vector.tensor_scalar(hwf[:], wq, 0.5, -0.5, Alu.mult,
                                        Alu.add)
                hwi = dpool.tile([128, 1], i16, tag="hwi")
                nc.vector.tensor_scalar(hwi[:], hwf[:], 2.6, None, Alu.max)
                hlf = dpool.tile([128, 1], fp32, tag="hlf")
                nc.vector.tensor_scalar(hlf[:], lq, 0.5, -0.5, Alu.mult,
                                        Alu.add)
                hli = dpool.tile([128, 1], i16, tag="hli")
                nc.vector.tensor_scalar(hli[:], hlf[:], 2.6, None, Alu.max)
                xmini = dpool.tile([128, 1], i16, tag="xmini")
                nc.vector.tensor_tensor(xmini[:], cxi[:], hwi[:],
                                        Alu.subtract)
                ymini = dpool.tile([128, 1], i16, tag="ymini")
                nc.vector.tensor_tensor(ymini[:], cyi[:], hli[:],
                                        Alu.subtract)
                lenxi = dpool.tile([128, 1], i16, tag="lenxi")
                nc.vector.tensor_scalar(lenxi[:], hwi[:], 2, 1, Alu.mult,
                                        Alu.add)
                lenyi = dpool.tile([128, 1], i16, tag="lenyi")
                nc.vector.tensor_scalar(lenyi[:], hli[:], 2, 1, Alu.mult,
                                        Alu.add)
                xminf = dpool.tile([128, 1], fp32, tag="xminf")
                nc.vector.tensor_copy(xminf[:], xmini[:])
                yminf = dpool.tile([128, 1], fp32, tag="yminf")
                nc.vector.tensor_copy(yminf[:], ymini[:])
                lenxf = dpool.tile([128, 1], fp32, tag="lenxf")
                nc.vector.tensor_copy(lenxf[:], lenxi[:])
                lenyf = dpool.tile([128, 1], fp32, tag="lenyf")
                nc.vector.tensor_copy(lenyf[:], lenyi[:])
                vw = dpool.tile([128, 1], fp32, tag="vw")
                nc.vector.tensor_scalar(vw[:], wq, 0.0, None, Alu.is_gt)
                vv = dpool.tile([128, 1], fp32, tag="vv")
                nc.vector.scalar_tensor_tensor(vv[:], lq, 0.0, vw[:],
                                               Alu.is_gt, Alu.logical_and)
                wsv = dpool.tile([128, 1], fp32, tag="wsv")
                nc.vector.tensor_tensor(wsv[:], w2n, vv[:], Alu.mult)
                wav = dpool.tile([128, 1], fp32, tag="wav")
                nc.vector.scalar_tensor_tensor(wav[:], zq, -1.0, wsv[:],
                                               Alu.mult, Alu.mult)

                # U (row indicator) / V (col) in bf16 via int16 compare
                tx = spool.tile([128, HO], i16, tag="tx")
                nc.vector.tensor_scalar(tx[:], iota16[:], xminf[:], None,
                                        Alu.subtract)
                U = spool.tile([128, HO], bf16, tag="U")
                nc.vector.tensor_scalar(U[:], tx[:].bitcast(u16),
                                        lenxf[:], None, Alu.is_lt)
                ty = spool.tile([128, HO], i16, tag="ty")
                nc.vector.tensor_scalar(ty[:], iota16[:], yminf[:], None,
                                        Alu.subtract)
                V0 = spool.tile([128, HO], bf16, tag="V0")
                nc.vector.tensor_scalar(V0[:], ty[:].bitcast(u16),
                                        lenyf[:], None, Alu.is_lt)
                Vs = spool.tile([128, HO], bf16, tag="Vs")
                nc.vector.tensor_scalar(Vs[:], V0[:], wsv[:], None, Alu.mult)
                Va = spool.tile([128, HO], bf16, tag="Va")
                nc.vector.tensor_scalar(Va[:], V0[:], wav[:], None, Alu.mult)

                # step1: out1 = F^T A (row resize), fp16; the two partition
                # chunks run in separate PSUM pools so they overlap.
                out1a = spool.tile([128, HO], fp16, tag="out1a")
                out1b = spool.tile([32, HO], fp16, tag="out1b")
                for msz, moff, o1, pool in ((128, 0, out1a, psS),
                                            (32, 128, out1b, psF)):
                    tag = "Ts" if pool is psS else "Tf"
                    p1 = pool.tile([128, HO], fp32, tag=tag)
                    for hs in BANKS:
                        nc.tensor.matmul(p1[0:msz, hs],
                                         F0[:, moff:moff + msz],
                                         A0h[:, hs], start=True, stop=False)
                        nc.tensor.matmul(p1[0:msz, hs],
                                         F1[:, moff:moff + msz],
                                         A1h[:, hs], start=False, stop=True)
                    nc.scalar.copy(o1[:], p1[0:msz, :])
                return {"U": U, "Vs": Vs, "Va": Va,
                        "out1a": out1a, "out1b": out1b}

            seq = [(rep, s) for rep in range(krep) for s in range(SPC)]
            cur = prep(*seq[0])
            for i, (rep, s) in enumerate(seq):
                nxt = None
                U, Vs, Va = cur["U"], cur["Vs"], cur["Va"]
                out1a, out1b = cur["out1a"], cur["out1b"]
                for m in range(5):
                    ms = slice(m * 128, (m + 1) * 128)
                    idx = ((rep * SPC + s) * 5) + m

                    Ts = psS.tile([128, HO], fp32, tag="Ts")
                    Ta = psA.tile([128, HO], fp32, tag="Ta")
                    for hs in BANKS:
                        nc.tensor.matmul(Ts[:, hs], U[:, ms], Vs[:, hs],
                                         start=True, stop=True)
                        nc.tensor.matmul(Ta[:, hs], U[:, ms], Va[:, hs],
                                         start=True, stop=True)

                    # deferred -z paint + loss for the previous tile: keeps
                    # the PE in-order queue from stalling on this tile's
                    # decode chain.
                    if pending is not None:
                        pTf, pZc, pidx = pending
                        for hs in BANKS:
                            nc.tensor.matmul(pTf[:, hs], idbf[:],
                                             pZc[:, hs], start=False,
                                             stop=True)
                        pdsq = dpool.tile([128, HO], bf16, tag="dsq")
                        nc.scalar.activation(
                            pdsq[:], pTf[:], ActF.Square,
                            accum_out=accbuf[:, pidx:pidx + 1])
                        pending = None

                    Tf = psF.tile([128, HO], fp32, tag="Tf")

                    # decode: Z = CA / 2^(top exponent of C), done as integer
                    # exponent subtraction on the raw fp32 bits.
                    # E1b = max(exp-bits(C) - bias, 0): uncovered pixels
                    # (C = 0) then decode to exactly 0.
                    E1i = dpool.tile([128, HO], i32, tag="E1i")
                    nc.vector.tensor_scalar(E1i[:], Ts[:].bitcast(i32),
                                            2139095040, None,
                                            Alu.bitwise_and)
                    E1b = dpool.tile([128, HO], i32, tag="E1b")
                    nc.gpsimd.tensor_scalar(E1b[:], E1i[:], 1065353216,
                                            0, Alu.subtract, Alu.max)
                    Zb = dpool.tile([128, HO], i32, tag="Zb")
                    nc.vector.tensor_tensor(Zb[:], Ta[:].bitcast(i32),
                                            E1b[:], Alu.subtract)
                    Zc = dpool.tile([128, HO], bf16, tag="Zc")
                    nc.gpsimd.tensor_scalar(Zc[:], Zb[:].bitcast(fp32),
                                            -2.0, 2.0, Alu.max, Alu.min)

                    # feat resize into Tf; -z paint + loss deferred.
                    for hs in BANKS:
                        nc.tensor.matmul(Tf[:, hs], out1a[:, ms],
                                         A0h[:, hs], start=True, stop=False)
                        nc.tensor.matmul(Tf[:, hs], out1b[:, ms],
                                         A1h[:, hs], start=False, stop=False)
                    pending = (Tf, Zc, idx)

                    if m == 1 and i + 1 < len(seq):
                        nxt = prep(*seq[i + 1])
                cur = nxt

            if pending is not None:
                pTf, pZc, pidx = pending
                for hs in BANKS:
                    nc.tensor.matmul(pTf[:, hs], idbf[:], pZc[:, hs],
                                     start=False, stop=True)
                pdsq = dpool.tile([128, HO], bf16, tag="dsq")
                nc.scalar.activation(pdsq[:], pTf[:], ActF.Square,
                                     accum_out=accbuf[:, pidx:pidx + 1])
                pending = None

            # ---- final reduction ----
            tot = cpool.tile([128, 1], fp32, tag="tot")
            nc.vector.tensor_reduce(
                tot[:], accbuf[:, 0:krep * SPC * 5],
                mybir.AxisListType.X, Alu.add)
            if krep > 1:
                nc.vector.tensor_scalar(tot[:], tot[:], 1.0 / krep, None,
                                        Alu.mult)
            pfin = psA.tile([128, HO], fp32, tag="Ta")
            nc.tensor.matmul(pfin[0:1, 0:1], tot[:], ones_t[:],
                             start=True, stop=True)
            res = cpool.tile([1, 1], fp32, tag="res")
            nc.scalar.copy(res[:], pfin[0:1, 0:1])
            nc.sync.dma_start(out_d.ap(), res[:])

    nc.compile()
    return nc


def _get_nc(krep=1):
    key = ("nc", krep)
    if key not in _CACHE:
        _CACHE[key] = _build(krep)
    return _CACHE[key]


def run_cores(feat, gt_bboxes, krep=1):
    """Run the SPMD kernel; returns list of per-core sum-of-squared-diffs."""
    from concourse.bass_utils import run_bass_kernel_spmd
    nc = _get_nc(krep)
    amat = _resize_matrix()
    iden = np.eye(128, dtype=np.float32)
    feat = np.asarray(feat, dtype=np.float32)
    gt = np.ascontiguousarray(np.asarray(gt_bboxes, dtype=np.float32))
    feat16 = feat.astype(np.float16)
    in_maps = []
    for i in range(NCORES):
        sl = slice(i * SPC, (i + 1) * SPC)
        in_maps.append({
            "feat": np.ascontiguousarray(feat16[sl, 0]),
            "boxes": np.ascontiguousarray(gt[sl]),
            "amat": amat,
            "iden": iden,
        })
    res = run_bass_kernel_spmd(nc, in_maps, core_ids=list(range(NCORES)))
    return [float(res.results[i]["out"][0, 0]) for i in range(NCORES)]


def kernel(feat, gt_bboxes):
    parts = run_cores(feat, gt_bboxes, krep=1)
    total = float(np.sum(np.asarray(parts, dtype=np.float64)))
    return np.asarray(np.float32(total / NPIX))
